# revision 21
# baseline (speedup 1.0000x reference)
"""Data-parallel 3x3 conv (NHWC 16x112x112x64, OHWI 64x3x3x64, pad=1, stride=1)
on 8 TRN2 NeuronCores via Bass/Tile.

v2 strategy (per core, 2 images) -- row-partition layout:
  - T1[pr, img*7168 + col*64 + c]: partition = padded row index pr (0..113;
    0/113 are zero pad rows, 114..127 garbage), free = (col, chan). Input is
    one SWDGE cast-DMA (f32->bf16) per column band with 112 large contiguous
    descriptors (rows are contiguous in NHWC DRAM).
  - Batched 128x128 xbar transposes (sync queue) T1 -> T2: each 128-wide free
    block of T1 = one column pair (E=col 2b, O=col 2b+1) x 64 chans becomes
    T2[(pos2, c), j] with j = imgbase + 128*b + pr. Zero gap blocks separate
    images so cross-block taps at image edges read zeros.
  - Conv as 9 matmuls per 512-position chunk accumulating in one PSUM bank
    (even/odd output columns in partitions 0:64 / 64:128):
      For row tap dy in 0..2 (free offset dy-1):
        mid  both += [[W(dy,1)|W(dy,0)],[W(dy,2)|W(dy,1)]]^T @ T2[:, j0+dy-1]
        so   odd  += W(dy,2)^T @ T2[0:64,  j0+dy-1+128]   (E of next pair)
        se   even += W(dy,0)^T @ T2[64:128, j0+dy-1-128]  (O of prev pair)
    so/se target disjoint row halves and co-execute on the PE.
  - Vector engine evacuates PSUM f32 -> T3 bf16 (scalar stays free).
  - Output xbar transposes (scalar queue) T3 -> T4[pr, (b, pos2*64+co)] in
    8-block bands emitted every 2 chunks; SWDGE cast-DMA T4 bf16 -> f32 NHWC
    output with 112 large contiguous descriptors per band.

Weights are host-packed (replicated tiny constant) into the lhsT tiles.
"""
import sys

sys.path.insert(0, "/opt/trn_rl_repo")

import ml_dtypes
import numpy as np

import concourse.bass as bass
import concourse.tile as tile
from concourse import bacc, mybir
from concourse.bass_utils import run_bass_kernel_spmd

# Problem geometry (hardcoded per spec)
N, H, W, C = 16, 112, 112, 64
NCORES = 8
NPER = N // NCORES          # images per core
BLK = 128                   # free elems per column-pair block (2 cols x 64 ch)
NB = W // 2                 # 56 column-pair blocks per image
FPI = NB * BLK              # 7168 free elems per image in T1/T3
ZW = 256                    # zero gap width in T2 around each image
F_IMG = (ZW, ZW + FPI + ZW)           # T2 start offset of each image's data
T2_LEN = ZW + FPI + ZW + FPI + ZW     # 15104
CHUNK = 512                 # positions per psum chunk (4 blocks)
CHUNKS_IMG = FPI // CHUNK   # 14
OBAND = 8                   # out-band size in blocks (1024 free positions)

f16 = mybir.dt.bfloat16  # 16-bit compute dtype (bf16: full-rate M=128 matmul)
f32 = mybir.dt.float32

IN_BANDS0 = [(0, 8), (8, 24), (24, 40), (40, 56)]
IN_BANDS1 = [(0, 28), (28, 56)]


def _conv_kernel(tc, x_ap, w_ap, z_ap, y_ap):
    nc = tc.nc
    with tc.tile_pool(name="wp", bufs=1) as wp, \
         tc.tile_pool(name="big", bufs=1) as big, \
         tc.tile_pool(name="ps", bufs=8, space="PSUM") as psp:

        wt = wp.tile([128, 576], f16)   # [3 dy x 128 mid cols] + [3 dy x 64 single cols]
        nc.scalar.dma_start(wt[:], w_ap)

        T1 = big.tile([128, NPER * FPI], f16)
        T2 = big.tile([128, T2_LEN], f16)
        T3 = big.tile([128, NPER * FPI], f16)
        T4 = big.tile([128, NPER * FPI], f16)
        T2v3 = T2[:].rearrange("p (a b) -> p a b", b=BLK)
        T4v3 = T4[:].rearrange("p (a b) -> p a b", b=BLK)

        # zero gap regions of T2 (read by taps at image/chunk borders)
        nc.vector.memset(T2[:, 0:ZW], 0)
        nc.vector.memset(T2[:, ZW + FPI: ZW + FPI + ZW], 0)
        nc.vector.memset(T2[:, T2_LEN - ZW:], 0)

        xt = x_ap.tensor
        yt = y_ap.tensor
        s_img, s_row = H * W * C, W * C   # DRAM strides (elements)

        # ---- input: SWDGE cast DMA f32->bf16 into T1 rows (pad rows 0/113
        # zeroed from z in one DMA), then batched 128x128 xbar transposes into
        # T2 (sync queue). Few DMA instructions total: the DMA-completion
        # semaphore pools are ~8 deep per queue, and exceeding them makes the
        # scheduler chain unrelated DMAs through sem-reuse guards.
        pitch = T1[:].ap[0][0]
        nc.gpsimd.dma_start(
            bass.AP(T1[:].tensor, 0, [[113 * pitch, 2], [1, NPER * FPI]]),
            z_ap[0:2, :])
        def in_dma(img, b0, b1):
            dram = bass.AP(xt, img * s_img + b0 * BLK,
                           [[s_row, H], [1, (b1 - b0) * BLK]])
            nc.gpsimd.dma_start(
                T1[1:113, img * FPI + b0 * BLK: img * FPI + b1 * BLK], dram)

        def in_xpose(img, b0, b1):
            a0 = F_IMG[img] // BLK
            nc.sync.dma_start(
                T2v3[:, a0 + b0: a0 + b1, :],
                T1[:, img * FPI + b0 * BLK: img * FPI + b1 * BLK],
                transpose=True)

        # img0's transposes are emitted before img1's big load so the
        # scheduler's per-queue tick coalescing can't chain them behind it
        for b in ((0, 0, 12), (0, 12, 28), (0, 28, 56)):
            in_dma(*b)
        for b in ((0, 0, 12), (0, 12, 28), (0, 28, 56)):
            in_xpose(*b)
        in_dma(1, 0, 56)
        in_xpose(1, 0, 28)
        in_xpose(1, 28, 56)

        # ---- compute (9 matmuls + vector evac per chunk); output bands of
        # 8 blocks emitted every 2 chunks (scalar xpose + gpsimd cast DMA out)
        T2v = T2[:]

        def emit_chunk(img, k):
            base = F_IMG[img] + k * CHUNK
            f3 = img * FPI + k * CHUNK
            ps = psp.tile([128, CHUNK], f32)
            for dy in range(3):
                off = base + dy - 1
                m = 128 * dy
                sgl = 384 + 64 * dy
                # merged mid K=128 M=128
                nc.tensor.matmul(ps[:, :], wt[:, m: m + 128],
                                 T2v[:, off: off + CHUNK],
                                 start=(dy == 0), stop=False,
                                 skip_group_check=True)
                # single odd (E rows @ +BLK): W(dy,2)
                nc.tensor.matmul(ps[64:128, :], wt[0:64, sgl: sgl + 64],
                                 T2v[0:64, off + BLK: off + BLK + CHUNK],
                                 start=False, stop=(dy == 2), skip_group_check=True)
                # single even (O rows @ -BLK): W(dy,0)
                nc.tensor.matmul(ps[0:64, :], wt[64:128, sgl: sgl + 64],
                                 T2v[64:128, off - BLK: off - BLK + CHUNK],
                                 start=False, stop=(dy == 2), skip_group_check=True)
            nc.vector.tensor_scalar_add(T3[:, f3: f3 + CHUNK], ps[:], 0.0)

        def emit_out_band(img, b0, b1, eng=None):
            nb = b1 - b0
            a0 = img * (FPI // BLK) + b0
            (eng or nc.scalar).dma_start(
                T4v3[:, a0: a0 + nb, :],
                T3[:, img * FPI + b0 * BLK: img * FPI + b1 * BLK],
                transpose=True)
            dram = bass.AP(yt, img * s_img + b0 * BLK,
                           [[s_row, H], [1, nb * BLK]])
            nc.gpsimd.dma_start(
                dram, T4[1:113, img * FPI + b0 * BLK: img * FPI + b1 * BLK])

        OUT_BANDS = [(6, 0, 28), (11, 28, 48), (13, 48, 56)]
        for img in range(NPER):
            bi = 0
            for k in range(CHUNKS_IMG):
                emit_chunk(img, k)
                while bi < len(OUT_BANDS) and OUT_BANDS[bi][0] == k:
                    _, b0, b1 = OUT_BANDS[bi]
                    emit_out_band(img, b0, b1)
                    bi += 1


_CACHE = {}


def _build():
    if "nc" in _CACHE:
        return _CACHE["nc"]
    nc = bacc.Bacc("TRN2", target_bir_lowering=False, debug=False,
                   num_devices=NCORES)
    x_d = nc.dram_tensor("x", [NPER * H * W * C], f32, kind="ExternalInput").ap()
    w_d = nc.dram_tensor("w", [128, 576], f16, kind="ExternalInput").ap()
    z_d = nc.dram_tensor("z", [2, NPER * FPI], f16, kind="ExternalInput").ap()
    y_d = nc.dram_tensor("y", [NPER * H * W * C], f32, kind="ExternalOutput").ap()
    with tile.TileContext(nc) as tc:
        _conv_kernel(tc, x_d, w_d, z_d, y_d)
    nc.compile()
    _CACHE["nc"] = nc
    return nc


def _pack_weights(kernels):
    # kernels: (C_OUT=64, 3, 3, C_IN=64) f32, OHWI.
    # Wt[dy][dx] = [ci, co] matrix
    wt = kernels.transpose(3, 1, 2, 0).astype(ml_dtypes.bfloat16)  # [ci, dy, dx, co]
    wpk = np.zeros((128, 576), ml_dtypes.bfloat16)
    for dy in range(3):
        m = 128 * dy
        wpk[0:64, m: m + 64] = wt[:, dy, 1]        # midE even-target
        wpk[0:64, m + 64: m + 128] = wt[:, dy, 0]  # midE odd-target
        wpk[64:128, m: m + 64] = wt[:, dy, 2]      # midO even-target
        wpk[64:128, m + 64: m + 128] = wt[:, dy, 1]  # midO odd-target
        sgl = 384 + 64 * dy
        wpk[0:64, sgl: sgl + 64] = wt[:, dy, 2]    # single odd (E rows)
        wpk[64:128, sgl: sgl + 64] = wt[:, dy, 0]  # single even (O rows)
    return wpk


def kernel(x, kernels, mode=None, _trace=False, **_):
    x = np.ascontiguousarray(np.asarray(x, dtype=np.float32))
    wpk = _pack_weights(np.asarray(kernels, dtype=np.float32))
    nc = _build()
    zer = np.zeros((2, NPER * FPI), ml_dtypes.bfloat16)
    in_maps = [{"x": x[i * NPER:(i + 1) * NPER].reshape(-1), "w": wpk, "z": zer}
               for i in range(NCORES)]
    res = run_bass_kernel_spmd(nc, in_maps, core_ids=list(range(NCORES)),
                               trace=_trace)
    out = np.concatenate(
        [res.results[i]["y"].reshape(NPER, H, W, C) for i in range(NCORES)], axis=0)
    if _trace:
        kernel.last_result = res
    return out.astype(np.float32)
